# revision 1
# baseline (speedup 1.0000x reference)
"""Sharded GQA attention (causal + packed-segment mask) for 8 Trainium2 NeuronCores.

Strategy
--------
* Core c handles batch b = c//4 and KV heads {2*(c%4), 2*(c%4)+1} (8 query
  heads per core); the sequence dim stays unsharded.
* decoder_segment_ids are sorted, so the segment mask makes attention
  block-diagonal over contiguous segment spans.  The host reads the actual
  ids, splits each batch into runs, and the device kernel does causal-only
  attention per segment.  The two batches' run structures are unioned
  (padded) so all 8 cores execute one SPMD program; padded "ghost" rows are
  masked with per-core additive mask tiles and ghost query columns produce
  garbage that the host discards on re-assembly.
* Per (segment, kv, t-block): S^T[s, (g,t)] tiles are built by PE matmuls
  (K-chunk stationary [d,s], Q^T moving [d, 4*128]); causal/ghost masking is
  an identity-stationary matmul accumulating a host-built additive mask
  (mask matmuls run as float32r -- products are exact 1*M/0*M, and fp32r
  streams 4x faster through the PE than fp32); exp runs on ScalarE straight
  out of PSUM (no max subtraction -- logits are bounded, fp32-safe); PV uses
  P^T tiles as stationary against V chunks padded to 130 columns with an
  appended ones column so the softmax denominator falls out of the same
  matmuls; the final normalize is a reciprocal + broadcast tensor_tensor
  multiply on DVE fused with the PSUM->SBUF copy.  QK/PV matmuls stay plain
  fp32: float32r (TF32-class rounding) measured 2.3x faster end-to-end but
  cost 1.9e-3 relative error vs the fp32 reference; fp32 keeps it at 6.5e-6.

Measured on the 8 axon-tunneled trn2 NeuronCores (For_i-looped timing):
  ~95.1 us per invocation, relative error 6.5e-06.
"""

import math

import numpy as np

B, T, NQ, NKV, D = 2, 1024, 32, 8, 128
G = NQ // NKV
NCORES = 8
KV_PER_CORE = NKV // (NCORES // B)
CHUNK = 128
NEG = -1.0e9
GRP = 1  # s-chunks per PSUM slab (1 bank, 4-deep pipelined)

_PROGRAM_CACHE = {}


# --------------------------------------------------------------------------
# host-side structure
# --------------------------------------------------------------------------

def _runs(seg_row):
    d = np.flatnonzero(np.diff(seg_row) != 0)
    starts = np.concatenate(([0], d + 1))
    ends = np.concatenate((d + 1, [len(seg_row)]))
    return [(int(s), int(e - s)) for s, e in zip(starts, ends)]


def _structure(ids):
    runs = [_runs(np.asarray(ids[b])) for b in range(B)]
    n_seg = max(len(r) for r in runs)
    L = [max((r[i][1] for r in runs if len(r) > i), default=0) for i in range(n_seg)]
    K = [math.ceil(l / CHUNK) for l in L]
    slots = set()
    for i in range(n_seg):
        if K[i] == 0:
            continue
        ghost = set()
        for b in range(B):
            lb = runs[b][i][1] if len(runs[b]) > i else 0
            for c in range(lb // CHUNK, K[i]):
                ghost.add(c)
        for j in range(K[i]):
            for c in range(j + 1):
                if c == j or c in ghost:
                    slots.add((i, c, j))
    slots = sorted(slots)
    segs = [i for i in range(n_seg) if K[i] > 0]
    slabs = [(i, kv_i, j) for i in segs for kv_i in range(KV_PER_CORE)
             for j in range(K[i])]
    chunks = [(i, kv_i, c) for i in segs for kv_i in range(KV_PER_CORE)
              for c in range(K[i])]
    return runs, L, K, slots, segs, slabs, chunks


def _prepare_core(core, q, k, v, runs, L, K, slots, segs, slabs, chunks):
    b = core // (NCORES // B)
    kv_heads = [KV_PER_CORE * (core % (NCORES // B)) + x for x in range(KV_PER_CORE)]
    rb = runs[b]

    def seg_info(i):
        if i < len(rb):
            return rb[i]
        return (0, 0)

    qT = np.zeros((D, len(slabs) * 4 * CHUNK), np.float32)
    for si, (i, kv_i, j) in enumerate(slabs):
        a, lb = seg_info(i)
        t0 = j * CHUNK
        n_real = min(CHUNK, lb - t0)
        if n_real > 0:
            for g in range(G):
                h = G * kv_heads[kv_i] + g
                blk = q[b, a + t0:a + t0 + n_real, h, :]  # [n_real, D]
                qT[:, si * 512 + g * CHUNK: si * 512 + g * CHUNK + n_real] = blk.T

    kT = np.zeros((D, len(chunks) * CHUNK), np.float32)
    vO = np.zeros((CHUNK, len(chunks) * 130), np.float32)
    for ci, (i, kv_i, c) in enumerate(chunks):
        a, lb = seg_info(i)
        s0 = c * CHUNK
        n_real = min(CHUNK, lb - s0)
        if n_real > 0:
            kvh = kv_heads[kv_i]
            kT[:, ci * CHUNK: ci * CHUNK + n_real] = k[b, a + s0:a + s0 + n_real, kvh, :].T
            vO[:n_real, ci * 130: ci * 130 + D] = v[b, a + s0:a + s0 + n_real, kvh, :]
            vO[:n_real, ci * 130 + D] = 1.0

    sr = np.arange(CHUNK)
    m4 = np.zeros((CHUNK, max(len(slots), 1) * 512), np.float32)
    for mi, (i, c, j) in enumerate(slots):
        _, lb = seg_info(i)
        srow = c * CHUNK + sr
        tcol = j * CHUNK + sr
        m = np.where((srow[:, None] > tcol[None, :]) | (srow[:, None] >= lb),
                     np.float32(NEG), np.float32(0.0))
        m4[:, mi * 512:(mi + 1) * 512] = np.tile(m, (1, G))

    return {"qT": qT, "kT": kT, "vO": vO, "m4": m4,
            "ident": np.eye(CHUNK, dtype=np.float32)}


def _assemble(outs, runs, L, K, slabs):
    full = np.zeros((B, T, NQ, D), np.float32)
    for core in range(NCORES):
        b = core // (NCORES // B)
        kv_heads = [KV_PER_CORE * (core % (NCORES // B)) + x
                    for x in range(KV_PER_CORE)]
        res = outs[core]  # [NSLAB, 128, 512]
        rb = runs[b]
        for si, (i, kv_i, j) in enumerate(slabs):
            if i >= len(rb):
                continue
            a, lb = rb[i]
            t0 = j * CHUNK
            n_real = min(CHUNK, lb - t0)
            if n_real <= 0:
                continue
            for g in range(G):
                h = G * kv_heads[kv_i] + g
                full[b, a + t0:a + t0 + n_real, h, :] = \
                    res[si, :n_real, g * CHUNK:g * CHUNK + D]
    return full


# --------------------------------------------------------------------------
# numpy emulation of the device schedule (debug/validation only)
# --------------------------------------------------------------------------

def _numpy_schedule(ins, L, K, slots, segs, slabs, chunks):
    slab_idx = {s: i for i, s in enumerate(slabs)}
    chunk_idx = {c: i for i, c in enumerate(chunks)}
    slot_idx = {s: i for i, s in enumerate(slots)}
    qT, kT, vO, m4 = ins["qT"], ins["kT"], ins["vO"], ins["m4"]
    out = np.zeros((len(slabs), CHUNK, 512), np.float32)
    for i in segs:
        for kv_i in range(KV_PER_CORE):
            for j in range(K[i]):
                si = slab_idx[(i, kv_i, j)]
                ot = np.zeros((CHUNK, G, 129), np.float32)
                for c in range(j + 1):
                    ci = chunk_idx[(i, kv_i, c)]
                    lhsT = kT[:, ci * CHUNK:(ci + 1) * CHUNK]          # [d, s]
                    rhs = qT[:, si * 512:(si + 1) * 512]               # [d, (g,t)]
                    S = lhsT.T @ rhs                                   # [s, (g,t)]
                    if (i, c, j) in slot_idx:
                        mi = slot_idx[(i, c, j)]
                        S = S + m4[:, mi * 512:(mi + 1) * 512]
                    P = np.exp(S)
                    vo = vO[:, ci * 130:ci * 130 + 129]                # [s, 129]
                    for g in range(G):
                        ot[:, g, :] += P[:, g * CHUNK:(g + 1) * CHUNK].T @ vo
                denom = ot[:, :, D:D + 1]
                with np.errstate(divide="ignore", invalid="ignore"):
                    norm = ot[:, :, :D] / denom
                out[si] = norm.reshape(CHUNK, G * D)
    return out


# --------------------------------------------------------------------------
# bass program
# --------------------------------------------------------------------------

def _build_program(L, K, slots, segs, slabs, chunks, loop_n=0, tiny_dma=False):
    import contextlib

    import concourse.bacc as bacc
    import concourse.bass as bass
    import concourse.tile as tile
    from concourse import mybir

    slab_idx = {s: i for i, s in enumerate(slabs)}
    chunk_idx = {c: i for i, c in enumerate(chunks)}
    slot_idx = {s: i for i, s in enumerate(slots)}
    f32 = mybir.dt.float32
    f32r = mybir.dt.float32r

    nc = bacc.Bacc()
    qT_d = nc.dram_tensor("qT", [D, len(slabs) * 512], f32, kind="ExternalInput")
    kT_d = nc.dram_tensor("kT", [D, len(chunks) * CHUNK], f32, kind="ExternalInput")
    vO_d = nc.dram_tensor("vO", [CHUNK, len(chunks) * 130], f32, kind="ExternalInput")
    m4_d = nc.dram_tensor("m4", [CHUNK, max(len(slots), 1) * 512], f32r,
                          kind="ExternalInput")
    id_d = nc.dram_tensor("ident", [CHUNK, CHUNK], f32r, kind="ExternalInput")
    out_d = nc.dram_tensor("out", [len(slabs), CHUNK, 512], f32,
                           kind="ExternalOutput")

    def _dma(eng, out, in_):
        if tiny_dma:
            eng.dma_start(out=out[:1, :4], in_=in_[:1, :4])
        else:
            eng.dma_start(out=out, in_=in_)

    with tile.TileContext(nc) as tc:
        with tc.tile_pool(name="pin", bufs=1) as pin, \
             tc.tile_pool(name="pp", bufs=8) as pp, \
             tc.tile_pool(name="po", bufs=3) as po, \
             tc.tile_pool(name="psum_s", bufs=4, space="PSUM") as psum_s, \
             tc.tile_pool(name="psum_o", bufs=2, space="PSUM") as psum_o, \
             (tc.For_i(0, loop_n, 1) if loop_n else
              contextlib.nullcontext()):

            ident_t = pin.tile([CHUNK, CHUNK], f32r, tag="ident")
            _dma(nc.sync, ident_t[:], id_d[:])
            m4all_t = pin.tile([CHUNK, max(len(slots), 1) * 512], f32r,
                               tag="m4")
            _dma(nc.sync, m4all_t[:], m4_d[:])
            m4_t = {s: m4all_t[:, mi * 512:(mi + 1) * 512]
                    for mi, s in enumerate(slots)}

            # inputs, emitted in compute-consumption order so the first
            # segment's tiles land first and compute starts early
            kT_t = {}
            vO_t = {}
            qT_t = {}
            for i in segs:
                for kv_i in range(KV_PER_CORE):
                    ci0 = chunk_idx[(i, kv_i, 0)]
                    kk = K[i]
                    kt = pin.tile([D, kk * CHUNK], f32, tag=f"kT_{i}_{kv_i}")
                    _dma(nc.sync, kt[:], kT_d[:, ci0 * CHUNK:(ci0 + kk) * CHUNK])
                    kT_t[(i, kv_i)] = kt
                    vt = pin.tile([CHUNK, kk * 130], f32, tag=f"vO_{i}_{kv_i}")
                    _dma(nc.sync, vt[:], vO_d[:, ci0 * 130:(ci0 + kk) * 130])
                    vO_t[(i, kv_i)] = vt
                    si0 = slab_idx[(i, kv_i, 0)]
                    qt = pin.tile([D, kk * 512], f32, tag=f"qT_{i}_{kv_i}")
                    _dma(nc.sync, qt[:], qT_d[:, si0 * 512:(si0 + kk) * 512])
                    for j in range(kk):
                        qT_t[(i, kv_i, j)] = qt[:, j * 512:(j + 1) * 512]

            for i in segs:
                for kv_i in range(KV_PER_CORE):
                    kt = kT_t[(i, kv_i)]
                    vt = vO_t[(i, kv_i)]
                    kk = K[i]
                    ostage = po.tile([CHUNK, kk * 512], f32,
                                     tag=f"os_{i}_{kv_i}", bufs=2)
                    for j in range(kk):
                        qt = qT_t[(i, kv_i, j)]
                        # two 1-bank output tiles (2 heads each) -> can
                        # double-buffer across j iterations
                        ot = [psum_o.tile([CHUNK, 2, 132], f32, tag=f"ot{h}",
                                          name=f"ot{h}")
                              for h in range(2)]
                        pts = []
                        for g0 in range(0, j + 1, GRP):
                            grp = list(range(g0, min(g0 + GRP, j + 1)))
                            slab = psum_s.tile([CHUNK, GRP, 512], f32, tag="slab")
                            for gi, c in enumerate(grp):
                                lhsT = kt[:, c * CHUNK:(c + 1) * CHUNK]
                                masked = (i, c, j) in slot_idx
                                nc.tensor.matmul(
                                    slab[:, gi, :], lhsT, qt,
                                    start=True, stop=not masked)
                                if masked:
                                    nc.tensor.matmul(
                                        slab[:, gi, :], ident_t[:],
                                        m4_t[(i, c, j)],
                                        start=False, stop=True)
                            pt = pp.tile([CHUNK, GRP, 512], f32, tag="pt")
                            nc.scalar.activation(
                                out=pt[:, :len(grp), :],
                                in_=slab[:, :len(grp), :],
                                func=mybir.ActivationFunctionType.Exp)
                            pts.append(pt)
                        for c in range(j + 1):
                            pt = pts[c // GRP]
                            psl = pt[:, c % GRP, :]
                            vsl = vt[:, c * 130:(c + 1) * 130]
                            for g in range(G):
                                # each ot bank holds two heads but forms ONE
                                # accumulation group: start clears has_written
                                # bank-wide, so only the first matmul into the
                                # bank starts and only the last one stops
                                nc.tensor.matmul(
                                    ot[g // 2][:, g % 2, 0:130],
                                    psl[:, g * CHUNK:(g + 1) * CHUNK],
                                    vsl,
                                    start=(c == 0 and g % 2 == 0),
                                    stop=(c == j and g % 2 == 1))
                        recip = po.tile([CHUNK, G], f32, tag="recip")
                        osl = ostage[:, j * 512:(j + 1) * 512]
                        for h in range(2):
                            rh = recip[:, 2 * h:2 * h + 2]
                            nc.vector.reciprocal(out=rh, in_=ot[h][:, :, D])
                            recip_b = bass.AP(
                                tensor=rh.tensor, offset=rh.offset,
                                ap=[rh.ap[0], rh.ap[1], [0, D]])
                            nc.vector.tensor_mul(
                                out=osl[:, 2 * h * 128:(2 * h + 2) * 128]
                                    .rearrange("p (g d) -> p g d", g=2),
                                in0=ot[h][:, :, 0:D],
                                in1=recip_b)
                    si0 = slab_idx[(i, kv_i, 0)]
                    if tiny_dma:
                        nc.sync.dma_start(out=out_d[si0][:1, :4],
                                          in_=ostage[:1, :4])
                    else:
                        nc.sync.dma_start(
                            out=out_d[si0:si0 + kk].rearrange("k p c -> p k c"),
                            in_=ostage[:].rearrange("p (k c) -> p k c", k=kk))

    nc.finalize()
    return nc


# --------------------------------------------------------------------------
# entry point
# --------------------------------------------------------------------------

def kernel(query, key, value, decoder_segment_ids, _trace=False, _numpy=False):
    query = np.asarray(query, np.float32)
    key = np.asarray(key, np.float32)
    value = np.asarray(value, np.float32)
    ids = np.asarray(decoder_segment_ids)
    # the block-diagonal decomposition relies on segment ids being sorted
    # (contiguous segments), as setup_inputs guarantees
    assert np.all(np.diff(ids.astype(np.int64), axis=-1) >= 0)

    runs, L, K, slots, segs, slabs, chunks = _structure(ids)
    core_ins = [_prepare_core(c, query, key, value, runs, L, K, slots,
                              segs, slabs, chunks) for c in range(NCORES)]

    if _numpy:
        outs = [_numpy_schedule(ci, L, K, slots, segs, slabs, chunks)
                for ci in core_ins]
        return _assemble(outs, runs, L, K, slabs)

    from concourse.bass_utils import run_bass_kernel_spmd

    cache_key = (tuple(L), tuple(slots))
    if cache_key not in _PROGRAM_CACHE:
        _PROGRAM_CACHE[cache_key] = _build_program(L, K, slots, segs, slabs,
                                                   chunks)
    nc = _PROGRAM_CACHE[cache_key]

    in_maps = [{k_: v_ for k_, v_ in ci.items()} for ci in core_ins]
    res = run_bass_kernel_spmd(nc, in_maps, list(range(NCORES)), trace=_trace)
    outs = [res.results[c]["out"] for c in range(NCORES)]
    full = _assemble(outs, runs, L, K, slabs)
    if _trace:
        return full, res
    return full



# revision 6
# speedup vs baseline: 1.7445x; 1.7445x over previous
"""Sharded GQA attention (causal + packed-segment mask) for 8 Trainium2 NeuronCores.

Strategy (v2)
-------------
* Core c handles batch b = c//4 and KV heads {2*(c%4), 2*(c%4)+1} (8 query
  heads per core); the sequence dim stays unsharded.
* decoder_segment_ids are sorted, so attention is block-diagonal over
  contiguous segments; the device kernel does causal-only attention per
  segment over 128-wide chunks.  The two batches' run structures are
  unioned so all 8 cores execute one SPMD program.
* dtypes: QK matmuls run float32r (TF32-class, 1 col/cycle at >=256 moving
  cols) or float16 (qdt config); P (post-exp) and V are bf16 so the
  130-col PV matmuls stream 1 col/cycle; output is bf16 (host upcasts).
* No mask matmuls: ghost rows/columns self-neutralise (zero K rows give
  S=0 -> P=1, but the matching V rows and ones-column are zero), so only
  the causal mask inside each diagonal 128x128 block is needed.  It is a
  single shared bf16 0/1 tile applied post-exp with one tensor_tensor
  multiply per diagonal chunk, split between DVE and GPSIMD.
* Q is packed host-side to only-real columns; QK, exp, normalize and the
  output DMA are all trimmed to real columns.
* exp runs once per slab (t-block) over a [128, (j+1), 4*nr] PSUM slab
  (chunk-per-bank); softmax denominators fall out of the PV matmuls via a
  bf16 ones-column appended to V; the normalize is one 4D broadcast
  tensor_mul per slab on DVE, writing bf16 staging that DMAs straight out.
"""

import math

import numpy as np
import ml_dtypes

B, T, NQ, NKV, D = 2, 1024, 32, 8, 128
G = NQ // NKV
NCORES = 8
KV_PER_CORE = NKV // (NCORES // B)
CHUNK = 128
BF16 = ml_dtypes.bfloat16

QDT = "f32r"          # "f32r" or "f16" for the QK matmul dtype
MASK_GP_FRAC = 0.5    # fraction of diag-mask multiplies routed to GPSIMD

_PROGRAM_CACHE = {}


# --------------------------------------------------------------------------
# host-side structure
# --------------------------------------------------------------------------

def _runs(seg_row):
    d = np.flatnonzero(np.diff(seg_row) != 0)
    starts = np.concatenate(([0], d + 1))
    ends = np.concatenate((d + 1, [len(seg_row)]))
    return [(int(s), int(e - s)) for s, e in zip(starts, ends)]


def _structure(ids):
    runs = [_runs(np.asarray(ids[b])) for b in range(B)]
    n_seg = max(len(r) for r in runs)
    L = [max((r[i][1] for r in runs if len(r) > i), default=0) for i in range(n_seg)]
    K = [math.ceil(l / CHUNK) for l in L]
    segs = [i for i in range(n_seg) if K[i] > 0]
    slabs = [(i, kv_i, j) for i in segs for kv_i in range(KV_PER_CORE)
             for j in range(K[i])]
    chunks = [(i, kv_i, c) for i in segs for kv_i in range(KV_PER_CORE)
              for c in range(K[i])]
    # real (non-ghost) q columns of slab (i, kv_i, j), from the union lengths
    nr = {(i, kv_i, j): min(CHUNK, L[i] - j * CHUNK)
          for (i, kv_i, j) in slabs}
    qbase = {}
    acc = 0
    for s in slabs:
        qbase[s] = acc
        acc += G * nr[s]
    return runs, L, K, segs, slabs, chunks, nr, qbase, acc


def _prepare_core(core, q, k, v, runs, L, K, segs, slabs, chunks, nr, qbase,
                  qcols, qdt=QDT):
    b = core // (NCORES // B)
    kv_heads = [KV_PER_CORE * (core % (NCORES // B)) + x for x in range(KV_PER_CORE)]
    rb = runs[b]
    np_qdt = np.float32 if qdt == "f32r" else np.float16

    def seg_info(i):
        if i < len(rb):
            return rb[i]
        return (0, 0)

    qT = np.zeros((D, qcols), np_qdt)
    for s in slabs:
        i, kv_i, j = s
        a, lb = seg_info(i)
        t0 = j * CHUNK
        n_real = min(nr[s], max(lb - t0, 0))
        if n_real > 0:
            base = qbase[s]
            for g in range(G):
                h = G * kv_heads[kv_i] + g
                blk = q[b, a + t0:a + t0 + n_real, h, :]  # [n_real, D]
                qT[:, base + g * nr[s]: base + g * nr[s] + n_real] = blk.T

    kT = np.zeros((D, len(chunks) * CHUNK), np_qdt)
    vO = np.zeros((CHUNK, len(chunks) * 130), BF16)
    for ci, (i, kv_i, c) in enumerate(chunks):
        a, lb = seg_info(i)
        s0 = c * CHUNK
        n_real = min(CHUNK, lb - s0)
        if n_real > 0:
            kvh = kv_heads[kv_i]
            kT[:, ci * CHUNK: ci * CHUNK + n_real] = \
                k[b, a + s0:a + s0 + n_real, kvh, :].T.astype(np_qdt)
            vO[:n_real, ci * 130: ci * 130 + D] = \
                v[b, a + s0:a + s0 + n_real, kvh, :].astype(BF16)
            vO[:n_real, ci * 130 + D] = BF16(1.0)

    sr = np.arange(CHUNK)
    keep = (sr[:, None] <= sr[None, :]).astype(np.float32)  # keep t >= s
    mask = np.concatenate([keep] * G, axis=1).astype(BF16)  # [s, g*128 + t]

    return {"qT": qT, "kT": kT, "vO": vO, "mask": mask}


def _assemble(outs, runs, slabs, nr):
    full = np.zeros((B, T, NQ, D), np.float32)
    for core in range(NCORES):
        b = core // (NCORES // B)
        kv_heads = [KV_PER_CORE * (core % (NCORES // B)) + x
                    for x in range(KV_PER_CORE)]
        res = outs[core]  # [NSLAB, 128, 512] bf16
        rb = runs[b]
        for si, (i, kv_i, j) in enumerate(slabs):
            if i >= len(rb):
                continue
            a, lb = rb[i]
            t0 = j * CHUNK
            n_real = min(CHUNK, lb - t0)
            if n_real <= 0:
                continue
            for g in range(G):
                h = G * kv_heads[kv_i] + g
                full[b, a + t0:a + t0 + n_real, h, :] = \
                    res[si, :n_real, g * CHUNK:g * CHUNK + D].astype(np.float32)
    return full


# --------------------------------------------------------------------------
# numpy emulation of the device schedule (debug/validation only)
# --------------------------------------------------------------------------

def _numpy_schedule(ins, L, K, segs, slabs, chunks, nr, qbase):
    chunk_idx = {c: i for i, c in enumerate(chunks)}
    qT = ins["qT"].astype(np.float32)
    kT = ins["kT"].astype(np.float32)
    vO = ins["vO"].astype(np.float32)
    mask = ins["mask"].astype(np.float32)
    out = np.zeros((len(slabs), CHUNK, G * CHUNK), BF16)
    for si, (i, kv_i, j) in enumerate(slabs):
        n = nr[(i, kv_i, j)]
        qt = qT[:, qbase[(i, kv_i, j)]: qbase[(i, kv_i, j)] + G * n]  # [d, 4n]
        ot = np.zeros((CHUNK, G, 130), np.float32)
        for c in range(j + 1):
            ci = chunk_idx[(i, kv_i, c)]
            lhsT = kT[:, ci * CHUNK:(ci + 1) * CHUNK]          # [d, s]
            S = lhsT.T @ qt                                    # [s, 4n]
            P = np.exp(S)
            if c == j:
                m = mask[:, :n]                                # [s, n]
                P = P * np.concatenate([m] * G, axis=1)
            P = P.astype(BF16).astype(np.float32)
            vo = vO[:, ci * 130:ci * 130 + 130]                # [s, 130]
            for g in range(G):
                ot[:n, g, :] += P[:, g * n:(g + 1) * n].T @ vo
        den = ot[:, :, D]
        with np.errstate(divide="ignore", invalid="ignore"):
            recip = 1.0 / den
            norm = ot[:, :, :D] * recip[:, :, None]
        out[si, :, :] = norm.reshape(CHUNK, G * D).astype(BF16)
    return out


# --------------------------------------------------------------------------
# bass program
# --------------------------------------------------------------------------

def _build_program(L, K, segs, slabs, chunks, nr, qbase, qcols, qdt=QDT,
                   loop_n=0):
    import contextlib

    import concourse.bacc as bacc
    import concourse.bass as bass
    import concourse.tile as tile
    from concourse import mybir

    chunk_idx = {c: i for i, c in enumerate(chunks)}
    f32 = mybir.dt.float32
    bf16 = mybir.dt.bfloat16
    mm_dt = mybir.dt.float32r if qdt == "f32r" else mybir.dt.float16
    maxK = max(K[i] for i in segs)
    CG = min(maxK, 3)  # chunks per PSUM slab group (bank each)
    nslab = len(slabs)
    nchunk = len(chunks)

    nc = bacc.Bacc()
    qT_d = nc.dram_tensor("qT", [D, qcols], mm_dt, kind="ExternalInput")
    kT_d = nc.dram_tensor("kT", [D, nchunk * CHUNK], mm_dt, kind="ExternalInput")
    vO_d = nc.dram_tensor("vO", [CHUNK, nchunk * 130], bf16, kind="ExternalInput")
    mask_d = nc.dram_tensor("mask", [CHUNK, G * CHUNK], bf16, kind="ExternalInput")
    out_d = nc.dram_tensor("out", [nslab, CHUNK, G * CHUNK], bf16,
                           kind="ExternalOutput")

    with tile.TileContext(nc) as tc:
        with tc.tile_pool(name="pin", bufs=2) as pin, \
             tc.tile_pool(name="pp", bufs=3) as pp, \
             tc.tile_pool(name="po", bufs=2) as po, \
             tc.tile_pool(name="psum_s", bufs=2, space="PSUM") as psum_s, \
             tc.tile_pool(name="psum_o", bufs=1, space="PSUM") as psum_o, \
             (tc.For_i(0, loop_n, 1) if loop_n else
              contextlib.nullcontext()):

            mask_t = pin.tile([CHUNK, G * CHUNK], bf16, tag="mask", bufs=1)
            nc.sync.dma_start(out=mask_t[:], in_=mask_d[:])
            # per-(i,kv) input tiles, emitted in consumption order: each
            # tile's last consumer finishes early in the next iteration's
            # timeline, so the For_i loop's n+1 DMAs overlap n's compute
            ikvs = sorted({(i, kv_i) for (i, kv_i, _) in slabs})
            kT_t, qT_t, vO_t = {}, {}, {}
            for (i, kv_i) in ikvs:
                kk = K[i]
                ci0 = chunk_idx[(i, kv_i, 0)]
                s0 = (i, kv_i, 0)
                qlen = sum(G * nr[(i, kv_i, j)] for j in range(kk))
                kt = pin.tile([D, kk * CHUNK], mm_dt, tag=f"kT_{i}_{kv_i}")
                nc.sync.dma_start(out=kt[:],
                                  in_=kT_d[:, ci0 * CHUNK:(ci0 + kk) * CHUNK])
                kT_t[(i, kv_i)] = kt
                qt = pin.tile([D, qlen], mm_dt, tag=f"qT_{i}_{kv_i}")
                nc.sync.dma_start(out=qt[:],
                                  in_=qT_d[:, qbase[s0]: qbase[s0] + qlen])
                qT_t[(i, kv_i)] = qt
                vt = pin.tile([CHUNK, kk * 130], bf16, tag=f"vO_{i}_{kv_i}")
                nc.sync.dma_start(out=vt[:],
                                  in_=vO_d[:, ci0 * 130:(ci0 + kk) * 130])
                vO_t[(i, kv_i)] = vt

            mask_idx = 0
            for si, (i, kv_i, j) in enumerate(slabs):
                n = nr[(i, kv_i, j)]
                fcols = G * n
                qoff = qbase[(i, kv_i, j)] - qbase[(i, kv_i, 0)]
                qt = qT_t[(i, kv_i)][:, qoff: qoff + fcols]
                ci0 = chunk_idx[(i, kv_i, 0)]

                # ---- QK + exp, in groups of <=CG chunks per PSUM slab ----
                pts = []  # (pt_tile, c0, glen)
                for c0 in range(0, j + 1, CG):
                    glen = min(CG, j + 1 - c0)
                    slab = psum_s.tile([CHUNK, CG, G * CHUNK], f32, tag="slab")
                    for gi in range(glen):
                        c = c0 + gi
                        lhsT = kT_t[:, (ci0 + c) * CHUNK:(ci0 + c + 1) * CHUNK]
                        nc.tensor.matmul(slab[:, gi, 0:fcols], lhsT, qt,
                                         start=True, stop=True)
                    pt = pp.tile([CHUNK, CG * G * CHUNK], bf16, tag="pt")
                    nc.scalar.activation(
                        out=pt[:, 0:glen * fcols]
                            .rearrange("p (k c) -> p k c", k=glen),
                        in_=slab[:, 0:glen, 0:fcols],
                        func=mybir.ActivationFunctionType.Exp)
                    pts.append((pt, c0, glen))

                # ---- causal mask on the diagonal chunk (post-exp) ----
                pt_j, c0_j, _ = pts[-1]
                diag_off = (j - c0_j) * fcols
                diag = pt_j[:, diag_off: diag_off + fcols] \
                    .rearrange("p (g t) -> p g t", g=G)
                m_ap = bass.AP(tensor=mask_t.tensor, offset=mask_t.offset,
                               ap=[mask_t.ap[0], [0, G], [1, n]])
                eng = (nc.gpsimd if (mask_idx % 2 == 0 and MASK_GP_FRAC > 0)
                       else nc.vector)
                if MASK_GP_FRAC >= 1.0:
                    eng = nc.gpsimd
                elif MASK_GP_FRAC <= 0.0:
                    eng = nc.vector
                eng.tensor_mul(out=diag, in0=diag, in1=m_ap)
                mask_idx += 1

                # ---- PV (+ denominator via ones column) ----
                ot = psum_o.tile([CHUNK, 2, 512], f32, tag="ot")
                for c in range(j + 1):
                    pt, c0, _ = pts[c // CG]
                    poff = (c - c0) * fcols
                    vsl = vO_t[:, (ci0 + c) * 130:(ci0 + c) * 130 + 130]
                    for g in range(G):
                        lhsT = pt[:, poff + g * n: poff + (g + 1) * n]
                        nc.tensor.matmul(
                            ot[0:n, g // 2, (g % 2) * 132:(g % 2) * 132 + 130],
                            lhsT, vsl,
                            start=(c == 0 and g % 2 == 0),
                            stop=(c == j and g % 2 == 1))

                # ---- normalize (DVE): recip + broadcast multiply ----
                recip = po.tile([CHUNK, G], f32, tag="recip")
                den_ap = bass.AP(tensor=ot.tensor, offset=ot.offset + D,
                                 ap=[ot.ap[0], [512, 2], [132, 2]])
                r4 = bass.AP(tensor=recip.tensor, offset=recip.offset,
                             ap=[recip.ap[0], [2, 2], [1, 2]])
                nc.vector.reciprocal(out=r4, in_=den_ap)
                osl = po.tile([CHUNK, G * CHUNK], bf16, tag="osl")
                out_ap = bass.AP(tensor=osl.tensor, offset=osl.offset,
                                 ap=[osl.ap[0], [2 * D, 2], [D, 2], [1, D]])
                num_ap = bass.AP(tensor=ot.tensor, offset=ot.offset,
                                 ap=[ot.ap[0], [512, 2], [132, 2], [1, D]])
                r_b = bass.AP(tensor=recip.tensor, offset=recip.offset,
                              ap=[recip.ap[0], [2, 2], [1, 2], [0, D]])
                nc.vector.tensor_mul(out=out_ap, in0=num_ap, in1=r_b)
                nc.sync.dma_start(out=out_d[si][0:n, :], in_=osl[0:n, :])

    nc.finalize()
    return nc


# --------------------------------------------------------------------------
# entry point
# --------------------------------------------------------------------------

def kernel(query, key, value, decoder_segment_ids, _trace=False, _numpy=False,
           _qdt=QDT):
    query = np.asarray(query, np.float32)
    key = np.asarray(key, np.float32)
    value = np.asarray(value, np.float32)
    ids = np.asarray(decoder_segment_ids)
    # the block-diagonal decomposition relies on segment ids being sorted
    # (contiguous segments), as setup_inputs guarantees
    assert np.all(np.diff(ids.astype(np.int64), axis=-1) >= 0)

    runs, L, K, segs, slabs, chunks, nr, qbase, qcols = _structure(ids)
    core_ins = [_prepare_core(c, query, key, value, runs, L, K, segs, slabs,
                              chunks, nr, qbase, qcols, qdt=_qdt)
                for c in range(NCORES)]

    if _numpy:
        outs = [_numpy_schedule(ci, L, K, segs, slabs, chunks, nr, qbase)
                for ci in core_ins]
        return _assemble(outs, runs, slabs, nr)

    from concourse.bass_utils import run_bass_kernel_spmd

    cache_key = (tuple(L), _qdt)
    if cache_key not in _PROGRAM_CACHE:
        _PROGRAM_CACHE[cache_key] = _build_program(
            L, K, segs, slabs, chunks, nr, qbase, qcols, qdt=_qdt)
    nc = _PROGRAM_CACHE[cache_key]

    in_maps = [{k_: v_ for k_, v_ in ci.items()} for ci in core_ins]
    res = run_bass_kernel_spmd(nc, in_maps, list(range(NCORES)), trace=_trace)
    outs = [res.results[c]["out"] for c in range(NCORES)]
    full = _assemble(outs, runs, slabs, nr)
    if _trace:
        return full, res
    return full


# revision 28
# speedup vs baseline: 3.6685x; 2.1029x over previous
"""Sharded GQA attention (causal + packed-segment mask) for 8 Trainium2 NeuronCores.

Strategy (v4)
-------------
* Core c handles batch b = c//4 and KV heads {2*(c%4), 2*(c%4)+1} (8 query
  heads per core); the sequence dim stays unsharded.
* decoder_segment_ids are sorted, so attention is block-diagonal over
  contiguous segments; the device kernel does causal-only attention per
  segment over 128-wide chunks.  The two batches' run structures are
  unioned so all 8 cores execute one SPMD program.
* dtypes: QK matmuls run float16 (or float32r via qdt config); P (post-exp)
  and V are bf16 so the 130-col PV matmuls stream 1 col/cycle; output is
  bf16 (host upcasts).  Expected end-to-end rel err ~6.5e-3.
* No mask matmuls: ghost rows/columns self-neutralise (zero K rows give
  S=0 -> P=1, but the matching V rows and ones-column are zero), so only
  the causal mask inside each diagonal 128x128 block is needed.  It is a
  single shared bf16 0/1 tile applied post-exp with one tensor_tensor
  multiply per diagonal chunk, split between DVE and GPSIMD.
* Q is packed host-side to only-real columns; QK, exp and normalize are
  trimmed to real columns.
* exp runs once per slab (t-block) over a [128, (j+1), 4*nr] PSUM slab
  (chunk-per-bank); softmax denominators fall out of the PV matmuls via a
  bf16 ones-column appended to V; the normalize is one 4D broadcast
  tensor_mul per slab on DVE.
* DMA-issue overhead (~1.2us per DMA of SEQ+DGE time) dominates at this
  scale, so all per-(i,kv) inputs (K^T, packed Q^T, V) ride in ONE
  uint16-packed DMA with bitcast views, and each (i,kv)'s four output
  slabs leave in one DMA from a staging tile: 13 DMAs per iteration.
"""

import math

import numpy as np
import ml_dtypes

B, T, NQ, NKV, D = 2, 1024, 32, 8, 128
G = NQ // NKV
NCORES = 8
KV_PER_CORE = NKV // (NCORES // B)
CHUNK = 128
BF16 = ml_dtypes.bfloat16

QDT = "f16"           # "f32r" or "f16" for the QK matmul dtype
MASK_MODE = "pe"      # "pe": additive NEG mask matmul fused into the QK
                      # accumulation; "dve": 0/1 multiply post-exp
MASK_GP_FRAC = 0.72   # dve mode: fraction of mask multiplies on GPSIMD
NEG = -1.0e9
CG = 2                # chunks per PSUM slab tile (banks each)
SLAB_BUFS = 2         # psum_s pool buffers
OT_BUFS = 2           # psum_o pool buffers

_PROGRAM_CACHE = {}


# --------------------------------------------------------------------------
# host-side structure
# --------------------------------------------------------------------------

def _runs(seg_row):
    d = np.flatnonzero(np.diff(seg_row) != 0)
    starts = np.concatenate(([0], d + 1))
    ends = np.concatenate((d + 1, [len(seg_row)]))
    return [(int(s), int(e - s)) for s, e in zip(starts, ends)]


def _structure(ids):
    runs = [_runs(np.asarray(ids[b])) for b in range(B)]
    n_seg = max(len(r) for r in runs)
    L = [max((r[i][1] for r in runs if len(r) > i), default=0) for i in range(n_seg)]
    K = [math.ceil(l / CHUNK) for l in L]
    segs = [i for i in range(n_seg) if K[i] > 0]
    slabs = [(i, kv_i, j) for i in segs for kv_i in range(KV_PER_CORE)
             for j in range(K[i])]
    chunks = [(i, kv_i, c) for i in segs for kv_i in range(KV_PER_CORE)
              for c in range(K[i])]
    # real (non-ghost) q columns of slab (i, kv_i, j), from the union lengths
    nr = {(i, kv_i, j): min(CHUNK, L[i] - j * CHUNK)
          for (i, kv_i, j) in slabs}
    qbase = {}
    acc = 0
    for s in slabs:
        qbase[s] = acc
        acc += G * nr[s]
    return runs, L, K, segs, slabs, chunks, nr, qbase, acc


def _ikv_layout(K, slabs, chunks, nr, qbase):
    """Per-(i,kv) packed-input column layout (units: 2-byte elements)."""
    chunk_idx = {c: i for i, c in enumerate(chunks)}
    ikvs = sorted({(i, kv_i) for (i, kv_i, _) in slabs})
    lay = {}
    base = 0
    for (i, kv_i) in ikvs:
        kk = K[i]
        qlen = sum(G * nr[(i, kv_i, j)] for j in range(kk))
        kcols = kk * CHUNK
        vcols = kk * 130
        lay[(i, kv_i)] = dict(base=base, kcols=kcols, qlen=qlen, vcols=vcols,
                              ci0=chunk_idx[(i, kv_i, 0)], kk=kk)
        base += kcols + qlen + vcols
    return ikvs, lay, base


def _prepare_core(core, q, k, v, runs, L, K, segs, slabs, chunks, nr, qbase,
                  qcols, qdt=QDT):
    b = core // (NCORES // B)
    kv_heads = [KV_PER_CORE * (core % (NCORES // B)) + x for x in range(KV_PER_CORE)]
    rb = runs[b]
    np_qdt = np.float32 if qdt == "f32r" else np.float16

    def seg_info(i):
        if i < len(rb):
            return rb[i]
        return (0, 0)

    qT = np.zeros((D, qcols), np_qdt)
    for s in slabs:
        i, kv_i, j = s
        a, lb = seg_info(i)
        t0 = j * CHUNK
        n_real = min(nr[s], max(lb - t0, 0))
        if n_real > 0:
            base = qbase[s]
            for g in range(G):
                h = G * kv_heads[kv_i] + g
                blk = q[b, a + t0:a + t0 + n_real, h, :]  # [n_real, D]
                qT[:, base + g * nr[s]: base + g * nr[s] + n_real] = blk.T

    kT = np.zeros((D, len(chunks) * CHUNK), np_qdt)
    vO = np.zeros((CHUNK, len(chunks) * 130), BF16)
    for ci, (i, kv_i, c) in enumerate(chunks):
        a, lb = seg_info(i)
        s0 = c * CHUNK
        n_real = min(CHUNK, lb - s0)
        if n_real > 0:
            kvh = kv_heads[kv_i]
            kT[:, ci * CHUNK: ci * CHUNK + n_real] = \
                k[b, a + s0:a + s0 + n_real, kvh, :].T.astype(np_qdt)
            vO[:n_real, ci * 130: ci * 130 + D] = \
                v[b, a + s0:a + s0 + n_real, kvh, :].astype(BF16)
            vO[:n_real, ci * 130 + D] = BF16(1.0)

    sr = np.arange(CHUNK)
    if MASK_MODE == "pe":
        keep = np.where(sr[:, None] > sr[None, :], np.float32(NEG),
                        np.float32(0.0))  # additive: NEG where t < s
    else:
        keep = (sr[:, None] <= sr[None, :]).astype(np.float32)  # 0/1 keep
    mask = np.concatenate([keep] * G, axis=1).astype(BF16)  # [s, g*128 + t]

    return {"qT": qT, "kT": kT, "vO": vO, "mask": mask,
            "ident": np.eye(CHUNK, dtype=BF16)}


def _pack_core(ci, K, slabs, chunks, nr, qbase, qdt=QDT):
    """Build the device in_map from the logical per-core arrays."""
    ikvs, lay, total = _ikv_layout(K, slabs, chunks, nr, qbase)
    if qdt == "f16":
        inb = np.zeros((CHUNK, total), np.uint16)
        for ikv in ikvs:
            l = lay[ikv]
            b0 = l["base"]
            ci0, kk = l["ci0"], l["kk"]
            s0 = (ikv[0], ikv[1], 0)
            inb[:, b0:b0 + l["kcols"]] = \
                ci["kT"][:, ci0 * CHUNK:(ci0 + kk) * CHUNK].view(np.uint16)
            b1 = b0 + l["kcols"]
            inb[:, b1:b1 + l["qlen"]] = \
                ci["qT"][:, qbase[s0]: qbase[s0] + l["qlen"]].view(np.uint16)
            b2 = b1 + l["qlen"]
            inb[:, b2:b2 + l["vcols"]] = \
                ci["vO"][:, ci0 * 130:(ci0 + kk) * 130].view(np.uint16)
        return {"inb": inb, "mask": ci["mask"], "ident": ci["ident"]}
    return {"kT": ci["kT"], "qT": ci["qT"], "vO": ci["vO"],
            "mask": ci["mask"], "ident": ci["ident"]}


def _assemble(outs, runs, slabs, nr):
    full = np.zeros((B, T, NQ, D), np.float32)
    for core in range(NCORES):
        b = core // (NCORES // B)
        kv_heads = [KV_PER_CORE * (core % (NCORES // B)) + x
                    for x in range(KV_PER_CORE)]
        res = outs[core]  # [NSLAB, 128, 512] bf16
        rb = runs[b]
        for si, (i, kv_i, j) in enumerate(slabs):
            if i >= len(rb):
                continue
            a, lb = rb[i]
            t0 = j * CHUNK
            n_real = min(CHUNK, lb - t0)
            if n_real <= 0:
                continue
            for g in range(G):
                h = G * kv_heads[kv_i] + g
                full[b, a + t0:a + t0 + n_real, h, :] = \
                    res[si, :n_real, g * CHUNK:g * CHUNK + D].astype(np.float32)
    return full


# --------------------------------------------------------------------------
# numpy emulation of the device schedule (debug/validation only)
# --------------------------------------------------------------------------

def _numpy_schedule(ins, L, K, segs, slabs, chunks, nr, qbase):
    chunk_idx = {c: i for i, c in enumerate(chunks)}
    qT = ins["qT"].astype(np.float32)
    kT = ins["kT"].astype(np.float32)
    vO = ins["vO"].astype(np.float32)
    mask = ins["mask"].astype(np.float32)
    out = np.zeros((len(slabs), CHUNK, G * CHUNK), BF16)
    for si, (i, kv_i, j) in enumerate(slabs):
        n = nr[(i, kv_i, j)]
        qt = qT[:, qbase[(i, kv_i, j)]: qbase[(i, kv_i, j)] + G * n]  # [d, 4n]
        ot = np.zeros((CHUNK, G, 130), np.float32)
        for c in range(j + 1):
            ci = chunk_idx[(i, kv_i, c)]
            lhsT = kT[:, ci * CHUNK:(ci + 1) * CHUNK]          # [d, s]
            S = lhsT.T @ qt                                    # [s, 4n]
            m = np.concatenate([mask[:, :n]] * G, axis=1)      # [s, 4n]
            if MASK_MODE == "pe":
                if c == j:
                    S = S + m
                P = np.exp(S)
            else:
                P = np.exp(S)
                if c == j:
                    P = P * m
            P = P.astype(BF16).astype(np.float32)
            vo = vO[:, ci * 130:ci * 130 + 130]                # [s, 130]
            for g in range(G):
                ot[:n, g, :] += P[:, g * n:(g + 1) * n].T @ vo
        den = ot[:, :, D]
        with np.errstate(divide="ignore", invalid="ignore"):
            recip = 1.0 / den
            norm = ot[:, :, :D] * recip[:, :, None]
        out[si, :, :] = norm.reshape(CHUNK, G * D).astype(BF16)
    return out


# --------------------------------------------------------------------------
# bass program
# --------------------------------------------------------------------------

def _build_program(L, K, segs, slabs, chunks, nr, qbase, qcols, qdt=QDT,
                   loop_n=0, unroll=1):
    import contextlib

    import concourse.bacc as bacc
    import concourse.bass as bass
    import concourse.tile as tile
    from concourse import mybir

    f32 = mybir.dt.float32
    bf16 = mybir.dt.bfloat16
    u16 = mybir.dt.uint16
    f16pack = qdt == "f16"
    mm_dt = mybir.dt.float32r if qdt == "f32r" else mybir.dt.float16
    maxK = max(K[i] for i in segs)
    nslab = len(slabs)
    nchunk = len(chunks)
    ikvs, lay, packed_cols = _ikv_layout(K, slabs, chunks, nr, qbase)

    nc = bacc.Bacc()
    if f16pack:
        inb_d = nc.dram_tensor("inb", [CHUNK, packed_cols], u16,
                               kind="ExternalInput")
    else:
        qT_d = nc.dram_tensor("qT", [D, qcols], mm_dt, kind="ExternalInput")
        kT_d = nc.dram_tensor("kT", [D, nchunk * CHUNK], mm_dt,
                              kind="ExternalInput")
        vO_d = nc.dram_tensor("vO", [CHUNK, nchunk * 130], bf16,
                              kind="ExternalInput")
    mask_d = nc.dram_tensor("mask", [CHUNK, G * CHUNK], bf16,
                            kind="ExternalInput")
    ident_d = nc.dram_tensor("ident", [CHUNK, CHUNK], bf16,
                             kind="ExternalInput")
    out_d = nc.dram_tensor("out", [nslab, CHUNK, G * CHUNK], bf16,
                           kind="ExternalOutput")
    slab_idx = {s: i for i, s in enumerate(slabs)}

    with tile.TileContext(nc) as tc:
      with tc.tile_pool(name="pin", bufs=1) as pin, \
           tc.tile_pool(name="pp", bufs=3) as pp, \
           tc.tile_pool(name="po", bufs=2) as po, \
           tc.tile_pool(name="psum_s", bufs=SLAB_BUFS, space="PSUM") as psum_s, \
           tc.tile_pool(name="psum_o", bufs=OT_BUFS, space="PSUM") as psum_o:
        # loop-invariant: causal mask + identity, loaded once
        mask_t = pin.tile([CHUNK, G * CHUNK], bf16, tag="mask")
        nc.sync.dma_start(out=mask_t[:], in_=mask_d[:])
        ident_t = pin.tile([CHUNK, CHUNK], bf16, tag="ident")
        nc.sync.dma_start(out=ident_t[:], in_=ident_d[:])
        with (tc.For_i(0, loop_n, 1) if loop_n else contextlib.nullcontext()):
          for _it in range(max(1, unroll)):
            # one packed input DMA per (i,kv), in consumption order, so the
            # For_i loop's n+1 DMAs overlap iteration n's compute
            kT_t, qT_t, vO_t = {}, {}, {}
            for ikv in ikvs:
                l = lay[ikv]
                kk = l["kk"]
                if f16pack:
                    icols = l["kcols"] + l["qlen"] + l["vcols"]
                    it = pin.tile([CHUNK, icols], u16,
                                  tag=f"in_{ikv[0]}_{ikv[1]}")
                    nc.sync.dma_start(
                        out=it[:], in_=inb_d[:, l["base"]: l["base"] + icols])
                    kT_t[ikv] = it[:, 0:l["kcols"]].bitcast(mm_dt)
                    qT_t[ikv] = it[:, l["kcols"]: l["kcols"] + l["qlen"]] \
                        .bitcast(mm_dt)
                    vO_t[ikv] = it[:, l["kcols"] + l["qlen"]: icols] \
                        .bitcast(bf16)
                else:
                    ci0 = l["ci0"]
                    s0 = (ikv[0], ikv[1], 0)
                    kt = pin.tile([D, kk * CHUNK], mm_dt,
                                  tag=f"kT_{ikv[0]}_{ikv[1]}")
                    nc.sync.dma_start(
                        out=kt[:], in_=kT_d[:, ci0 * CHUNK:(ci0 + kk) * CHUNK])
                    kT_t[ikv] = kt[:]
                    qt = pin.tile([D, l["qlen"]], mm_dt,
                                  tag=f"qT_{ikv[0]}_{ikv[1]}")
                    nc.sync.dma_start(
                        out=qt[:], in_=qT_d[:, qbase[s0]: qbase[s0] + l["qlen"]])
                    qT_t[ikv] = qt[:]
                    vt = pin.tile([CHUNK, kk * 130], bf16,
                                  tag=f"vO_{ikv[0]}_{ikv[1]}")
                    nc.sync.dma_start(
                        out=vt[:], in_=vO_d[:, ci0 * 130:(ci0 + kk) * 130])
                    vO_t[ikv] = vt[:]

            # ---- software-pipelined wavefront over the (i,kv) streams ----
            # Streams are independent; stagger them by one j-step and emit
            # stage1 (QK+exp+mask) of step t before stage2 (PV+normalize)
            # of step t-1, so every engine's in-order queue always holds
            # dependency-resolved work.
            mask_state = {"idx": 0}
            ost_t = {}

            def stage1(i, kv_i, j):
                kt = kT_t[(i, kv_i)]
                n = nr[(i, kv_i, j)]
                fcols = G * n
                qoff = qbase[(i, kv_i, j)] - qbase[(i, kv_i, 0)]
                qt = qT_t[(i, kv_i)][:, qoff: qoff + fcols]
                m_ap = bass.AP(tensor=mask_t.tensor, offset=mask_t.offset,
                               ap=[mask_t.ap[0], [0, G], [1, n]])
                pts = []  # (pt_tile, c0, glen)
                for c0 in range(0, j + 1, CG):
                    glen = min(CG, j + 1 - c0)
                    slab = psum_s.tile([CHUNK, CG, G * CHUNK], f32,
                                       tag="slab")
                    for gi in range(glen):
                        c = c0 + gi
                        masked = MASK_MODE == "pe" and c == j
                        nc.tensor.matmul(
                            slab[:, gi, 0:fcols],
                            kt[:, c * CHUNK:(c + 1) * CHUNK], qt,
                            start=True, stop=not masked)
                        if masked:
                            # accumulate the additive NEG causal mask into
                            # the diagonal chunk's S (same PSUM group, no
                            # cross-engine hop; exp then emits exact zeros)
                            sl3 = slab[:, gi, 0:fcols] \
                                .rearrange("p (g t) -> p g t", g=G)
                            nc.tensor.matmul(sl3, ident_t[:], m_ap,
                                             start=False, stop=True)
                    pt = pp.tile([CHUNK, CG * G * CHUNK], bf16, tag="pt",
                                 bufs=8)
                    nc.scalar.activation(
                        out=pt[:, 0:glen * fcols]
                            .rearrange("p (k c) -> p k c", k=glen),
                        in_=slab[:, 0:glen, 0:fcols],
                        func=mybir.ActivationFunctionType.Exp)
                    pts.append((pt, c0, glen))

                if MASK_MODE == "dve":
                    # causal mask on the diagonal chunk (post-exp)
                    pt_j, c0_j, _ = pts[-1]
                    diag_off = (j - c0_j) * fcols
                    diag = pt_j[:, diag_off: diag_off + fcols] \
                        .rearrange("p (g t) -> p g t", g=G)
                    mi = mask_state["idx"]
                    gp_due = int(round((mi + 1) * MASK_GP_FRAC)) \
                        - int(round(mi * MASK_GP_FRAC))
                    eng = nc.gpsimd if gp_due else nc.vector
                    eng.tensor_mul(out=diag, in0=diag, in1=m_ap)
                    mask_state["idx"] = mi + 1
                return pts

            def stage2(i, kv_i, j, pts):
                kk = K[i]
                n = nr[(i, kv_i, j)]
                fcols = G * n
                vt = vO_t[(i, kv_i)]
                if j == 0:
                    ost = po.tile([CHUNK, kk * G * CHUNK], bf16,
                                  tag=f"ost_{i}_{kv_i}", bufs=2,
                                  name=f"ost_{i}_{kv_i}")
                    ost_t[(i, kv_i)] = ost
                ost = ost_t[(i, kv_i)]

                ot = psum_o.tile([CHUNK, 2, 512], f32, tag="ot")
                for c in range(j + 1):
                    pt, c0, _ = pts[c // CG]
                    poff = (c - c0) * fcols
                    vsl = vt[:, c * 130:c * 130 + 130]
                    for g in range(G):
                        nc.tensor.matmul(
                            ot[0:n, g // 2,
                               (g % 2) * 132:(g % 2) * 132 + 130],
                            pt[:, poff + g * n: poff + (g + 1) * n], vsl,
                            start=(c == 0 and g % 2 == 0),
                            stop=(c == j and g % 2 == 1))

                # normalize (DVE): recip + broadcast multiply into staging
                recip = po.tile([CHUNK, G], f32, tag="recip", bufs=4)
                den_ap = bass.AP(tensor=ot.tensor, offset=ot.offset + D,
                                 ap=[ot.ap[0], [512, 2], [132, 2]])
                r4 = bass.AP(tensor=recip.tensor, offset=recip.offset,
                             ap=[recip.ap[0], [2, 2], [1, 2]])
                nc.vector.reciprocal(out=r4, in_=den_ap)
                obase = j * G * CHUNK
                out_ap = bass.AP(tensor=ost.tensor,
                                 offset=ost.offset + obase,
                                 ap=[ost.ap[0], [2 * D, 2], [D, 2], [1, D]])
                num_ap = bass.AP(tensor=ot.tensor, offset=ot.offset,
                                 ap=[ot.ap[0], [512, 2], [132, 2], [1, D]])
                r_b = bass.AP(tensor=recip.tensor, offset=recip.offset,
                              ap=[recip.ap[0], [2, 2], [1, 2], [0, D]])
                nc.vector.tensor_mul(out=out_ap, in0=num_ap, in1=r_b)

                if j == kk - 1:
                    si0 = slab_idx[(i, kv_i, 0)]
                    nc.sync.dma_start(
                        out=out_d[si0:si0 + kk].rearrange("k p c -> p k c"),
                        in_=ost[:].rearrange("p (k c) -> p k c", k=kk))

            pending = []
            for t in range(len(ikvs) + maxK - 1):
                cur = []
                for g in range(len(ikvs)):
                    j = t - g
                    i, kv_i = ikvs[g]
                    if 0 <= j < K[i]:
                        cur.append((i, kv_i, j, stage1(i, kv_i, j)))
                for (i, kv_i, j, pts) in pending:
                    stage2(i, kv_i, j, pts)
                pending = cur
            for (i, kv_i, j, pts) in pending:
                stage2(i, kv_i, j, pts)

    nc.finalize()
    return nc


# --------------------------------------------------------------------------
# entry point
# --------------------------------------------------------------------------

def kernel(query, key, value, decoder_segment_ids, _trace=False, _numpy=False,
           _qdt=QDT):
    query = np.asarray(query, np.float32)
    key = np.asarray(key, np.float32)
    value = np.asarray(value, np.float32)
    ids = np.asarray(decoder_segment_ids)
    # the block-diagonal decomposition relies on segment ids being sorted
    # (contiguous segments), as setup_inputs guarantees
    assert np.all(np.diff(ids.astype(np.int64), axis=-1) >= 0)

    runs, L, K, segs, slabs, chunks, nr, qbase, qcols = _structure(ids)
    core_ins = [_prepare_core(c, query, key, value, runs, L, K, segs, slabs,
                              chunks, nr, qbase, qcols, qdt=_qdt)
                for c in range(NCORES)]

    if _numpy:
        outs = [_numpy_schedule(ci, L, K, segs, slabs, chunks, nr, qbase)
                for ci in core_ins]
        return _assemble(outs, runs, slabs, nr)

    from concourse.bass_utils import run_bass_kernel_spmd

    cache_key = (tuple(L), _qdt)
    if cache_key not in _PROGRAM_CACHE:
        _PROGRAM_CACHE[cache_key] = _build_program(
            L, K, segs, slabs, chunks, nr, qbase, qcols, qdt=_qdt)
    nc = _PROGRAM_CACHE[cache_key]

    in_maps = [_pack_core(ci, K, slabs, chunks, nr, qbase, qdt=_qdt)
               for ci in core_ins]
    res = run_bass_kernel_spmd(nc, in_maps, list(range(NCORES)), trace=_trace)
    outs = [res.results[c]["out"] for c in range(NCORES)]
    full = _assemble(outs, runs, slabs, nr)
    if _trace:
        return full, res
    return full
